# revision 30
# baseline (speedup 1.0000x reference)
"""Trainium2 Bass kernel for a dense transformer block (pre-LN, causal MHA + FFN).

Shapes (hardcoded): x [1024, 64, 384] fp32, 6 heads x 64, FFN hidden 1536.
Strategy: data-parallel over batch across 8 NeuronCores (128 seqs/core), no
collectives. Per core, one fused loop over segments of 8 token tiles
(16 sequences): LN1 -> QKV -> causal attention -> proj+residual -> LN2 ->
FFN+residual.

Precision: the large matmuls (QKV, attn-proj, FFN1, FFN2) run in fp8-e4m3
with DoubleRow perf mode (2 contraction rows per PE cell -> ~2x matmul
throughput). Weights are scaled by 8 host-side to stay in e4m3 normal range;
descales fold into existing activation scales (exp's score scale, relu's
bias trick) and fused (psum*s + residual) DVE ops, so no extra instructions.
Attention internals (scores, softmax, AV) stay bf16/fp32: same PE speed at
those shapes, and avoids DVE throughput penalties on 1-byte dtypes.

Engine balance (per 1024-token segment), tuned empirically on HW (the DVE is
the scarce engine; its queue stalls cost more than the cost model predicts):
PE does matmuls+transposes; ACT does exp, both LN normalizes (per-partition
affine with batched -mean*rstd bias), 3/4 of relu, the qk psum evictions and
half the other evictions; DVE does bn_stats, softmax normalize+mask, the
fused fp8-descale+residual adds, and the rest. GPSIMD only triggers weight
DMAs -- its compute ops measured catastrophically slow on HW. LN scalar
chains (ln/exp of variance) are pair-batched; x is DMA'd once (paired loads)
and stays resident for the residual; output stores are pair-batched.

Layout notes (contraction must sit on SBUF partitions for both operands):
 - xnF: LN1 output transposed to feature-major via PE transposes; serves as
   moving operand for q/k projections and stationary operand for v. fp8, with
   the 3 feature chunks at stride SW so DoubleRow pair slices [P, 2, w] work.
 - attention computes S^T = k @ q^T directly (scores transposed, [s, t]) so
   the softmax matrix is already stationary-ready for the AV matmul; the
   softmax denominator comes free as an extra 8.0-column in the v operand
   (v is scaled by 8 from the fp8 weight scaling; 8/8 cancels exactly).
 - softmax skips the max-subtraction: scores are O(1) by construction
   (LN'd activations times 0.02-scale weights), exp is safe in fp32.
"""

import os
import sys

import numpy as np

for _p in ("/opt/trn_rl_repo", os.path.expanduser("~/.axon_site/_ro/trn_rl_repo")):
    if os.path.isdir(_p) and _p not in sys.path:
        sys.path.insert(0, _p)

import ml_dtypes  # noqa: E402
import concourse.hw_specs as _hw_specs  # noqa: E402
import concourse.bacc as bacc  # noqa: E402
import concourse.tile as tile  # noqa: E402
from concourse import mybir  # noqa: E402
from concourse.bass_utils import run_bass_kernel_spmd  # noqa: E402

# Pin every activation function this kernel uses (Exp/Ln/Identity/Copy/Relu)
# to the one act table that contains them all (natural_log_exp_and_others).
# The default per-instruction table choice ping-pongs between tables, costing
# a ~1.3us table reload per switch on the ACT engine.
_ACT_PIN = {mybir.ActivationFunctionType.Exp, mybir.ActivationFunctionType.Ln,
            mybir.ActivationFunctionType.Identity,
            mybir.ActivationFunctionType.Copy,
            mybir.ActivationFunctionType.Relu}
_orig_get_tables = _hw_specs.get_activation_tables


def _pinned_tables(arch):
    out = {}
    for name, fns in _orig_get_tables(arch).items():
        out[name] = fns if name == "natural_log_exp_and_others" \
            else fns - _ACT_PIN
    return out


_hw_specs.get_activation_tables = _pinned_tables
bacc.get_activation_tables = _pinned_tables

BF16 = mybir.dt.bfloat16
FP8 = mybir.dt.float8e4
F32 = mybir.dt.float32
ACTF = mybir.ActivationFunctionType
ALU = mybir.AluOpType
DRM = mybir.MatmulPerfMode.DoubleRow

N_CORES = 8
B_FULL, T, C, H, D = 1024, 64, 384, 6, 64
J = 4 * C                       # 1536
B_LOC = B_FULL // N_CORES       # 128 sequences per core
NTOK = B_LOC * T                # 8192 tokens per core
P = 128
NT = NTOK // P                  # 64 token tiles (each tile = one pair of seqs)
SEG = 8                         # token tiles per fused segment
KC = C // P                     # 3 contraction chunks over C
JC = J // P                     # 12 chunks over FFN hidden
EPS = 1e-5
SCALE = D ** -0.5
WS = 8.0                        # fp8 weight scale
RWS = 1.0 / WS

_CACHE = {}
last_exec_time_ns = None


def _build(has_bv, has_bo, has_b2, nt=NT, loop_n=1):
    assert nt % SEG == 0 and (SEG * P) % 512 == 0
    nc = bacc.Bacc("TRN2", target_bir_lowering=False, debug=False)
    ntok = nt * P
    nseg = nt // SEG
    SW = SEG * P                # tokens per segment (1024)

    x_d = nc.dram_tensor("x", [ntok, C], F32, kind="ExternalInput").ap()
    wq_d = nc.dram_tensor("wq", [C, C], FP8, kind="ExternalInput").ap()
    wk_d = nc.dram_tensor("wk", [C, C], FP8, kind="ExternalInput").ap()
    wv_d = nc.dram_tensor("wv", [C, C], FP8, kind="ExternalInput").ap()
    wo_d = nc.dram_tensor("wo", [C, C], FP8, kind="ExternalInput").ap()
    w1_d = nc.dram_tensor("w1", [C, J], FP8, kind="ExternalInput").ap()
    w2_d = nc.dram_tensor("w2", [J, C], FP8, kind="ExternalInput").ap()
    bq_d = nc.dram_tensor("bq", [P, KC], F32, kind="ExternalInput").ap()
    bk_d = nc.dram_tensor("bk", [P, KC], F32, kind="ExternalInput").ap()
    bh_d = nc.dram_tensor("bh", [P, JC], F32, kind="ExternalInput").ap()
    bv_d = nc.dram_tensor("bv", [1, C], BF16, kind="ExternalInput").ap()
    bo_d = nc.dram_tensor("bo_r", [1, C], BF16, kind="ExternalInput").ap()
    b2_d = nc.dram_tensor("b2_r", [1, C], BF16, kind="ExternalInput").ap()
    id_d = nc.dram_tensor("ident", [P, P], BF16, kind="ExternalInput").ap()
    mk_d = nc.dram_tensor("maskt", [P, H * P], BF16, kind="ExternalInput").ap()
    out_d = nc.dram_tensor("out", [ntok, C], F32, kind="ExternalOutput").ap()

    with tile.TileContext(nc) as tc:
        with tc.tile_pool(name="singles", bufs=1) as sg, \
             tc.tile_pool(name="seg", bufs=2) as sgp, \
             tc.tile_pool(name="work", bufs=5) as wk, \
             tc.tile_pool(name="psum", bufs=1, space="PSUM") as ps:

            # ---- resident weights / constants ----
            # fp8 weights laid out [P, chunk, cols] so DoubleRow pair slices
            # [:, k:k+2, :] have pair-dim stride divisible by 16B.
            wq_sb = sg.tile([P, KC, C], FP8, name="wq")
            wk_sb = sg.tile([P, KC, C], FP8, name="wk")
            wv_sb = sg.tile([P, KC, C], FP8, name="wv")
            wo_sb = sg.tile([P, KC, C], FP8, name="wo")
            w1_sb = sg.tile([P, KC, J], FP8, name="w1")
            w2_sb = sg.tile([P, JC, C], FP8, name="w2")
            for k in range(KC):
                nc.gpsimd.dma_start(out=wq_sb[:, k, :],
                                    in_=wq_d[k * P:(k + 1) * P, :])
                nc.gpsimd.dma_start(out=wk_sb[:, k, :],
                                    in_=wk_d[k * P:(k + 1) * P, :])
            for k in range(KC):
                nc.gpsimd.dma_start(out=wv_sb[:, k, :],
                                    in_=wv_d[k * P:(k + 1) * P, :])
            for k in range(KC):
                nc.gpsimd.dma_start(out=wo_sb[:, k, :],
                                    in_=wo_d[k * P:(k + 1) * P, :])
                nc.gpsimd.dma_start(out=w1_sb[:, k, :],
                                    in_=w1_d[k * P:(k + 1) * P, :])
            for k in range(JC):
                nc.gpsimd.dma_start(out=w2_sb[:, k, :],
                                    in_=w2_d[k * P:(k + 1) * P, :])
            bq_sb = sg.tile([P, KC], F32)
            bk_sb = sg.tile([P, KC], F32)
            bh_sb = sg.tile([P, JC], F32)
            ident = sg.tile([P, P], BF16)
            maskt = sg.tile([P, H * P], BF16)
            nc.sync.dma_start(out=ident, in_=id_d)
            nc.scalar.dma_start(out=bq_sb, in_=bq_d)
            nc.scalar.dma_start(out=bk_sb, in_=bk_d)
            nc.scalar.dma_start(out=bh_sb, in_=bh_d)
            nc.scalar.dma_start(out=maskt, in_=mk_d)
            eps_sb = sg.tile([P, 1], F32)
            nc.vector.memset(eps_sb, EPS)
            ones1 = sg.tile([1, P], BF16)
            nc.vector.memset(ones1, 1.0)
            bv_sb = sg.tile([1, C], BF16)
            bo_sb = sg.tile([1, C], BF16)
            b2_sb = sg.tile([1, C], BF16)
            if has_bv:
                nc.sync.dma_start(out=bv_sb, in_=bv_d)
            if has_bo:
                nc.sync.dma_start(out=bo_sb, in_=bo_d)
            if has_b2:
                nc.sync.dma_start(out=b2_sb, in_=b2_d)

            def bass_strided(dstF, t):
                # [P, KC, 128] view of dstF hitting columns k*SW + t*128
                return dstF.rearrange("p (k w) -> p k w", k=KC)[
                    :, :, t * P:(t + 1) * P]

            def _copy(idx, out, in_):
                if idx % 2 == 0:
                    nc.scalar.copy(out=out, in_=in_)
                else:
                    nc.vector.tensor_copy(out=out, in_=in_)

            def ln_stats_pair(src_pair, st2, mv8, p):
                """bn stats of a [128, 2, C] fp32 pair -> mv8[:, 2p:2p+2, :].
                (two bn_stats: the op's free-dim HW limit is 512 < 2*C)"""
                for i in range(2):
                    nc.vector.bn_stats(out=st2[:, i, :], in_=src_pair[:, i, :])
                    nc.vector.bn_aggr(out=mv8[:, 2 * p + i, :],
                                      in_=st2[:, i, :])

            def ln_scal(mv8, rstd8, p):
                """Batched pair p: rstd[:, 2p:2p+2] = (var+eps)^-0.5 via
                exp(-0.5*ln(var+eps)) to stay in the pinned act table."""
                lnv = wk.tile([P, 2], F32, tag="lnlnv")
                nc.scalar.activation(out=lnv, in_=mv8[:, 2 * p:2 * p + 2, 1],
                                     func=ACTF.Ln, bias=eps_sb, scale=1.0)
                nc.scalar.activation(out=rstd8[:, 2 * p:2 * p + 2], in_=lnv,
                                     func=ACTF.Exp, bias=0.0, scale=-0.5)

            def tp_to_F(t, xn0, dstF, eng_off=0):
                tp = ps.tile([P, C], BF16, tag="big", bufs=2, name="tp")
                for k in range(KC):
                    nc.tensor.transpose(tp[:, k * P:(k + 1) * P],
                                        xn0[:, k * P:(k + 1) * P], ident)
                # one strided copy (with bf16 -> fp8 convert): chunk k lands
                # at column k*SW + t*128
                _copy(t + eng_off, bass_strided(dstF, t),
                      tp.rearrange("p (k c) -> p k c", c=P))

            def tp_pair_to_F(u, xn0a, xn0b, dstF, eng_off=0):
                # transposes for a PAIR of tiles into one [P, 768] bf16 psum
                # tile, evicted with ONE strided copy (halves evict op count)
                tp2 = ps.tile([P, KC * 2 * P], BF16, tag="big", bufs=2,
                              name="tp2")
                for tsub, xn0 in ((0, xn0a), (1, xn0b)):
                    for k in range(KC):
                        nc.tensor.transpose(
                            tp2[:, (k * 2 + tsub) * P:(k * 2 + tsub + 1) * P],
                            xn0[:, k * P:(k + 1) * P], ident)
                dstF_r = dstF.rearrange("p (k w) -> p k w", k=KC)
                _copy(u + eng_off,
                      dstF_r[:, :, 2 * u * P:(2 * u + 2) * P],
                      tp2.rearrange("p (k w) -> p k w", k=KC))

            ng = SW // 512

            def new_state(s):
                st_ = {"i0": s * SEG}
                st_["xnF"] = sgp.tile([P, KC * SW], FP8, tag="xnF",
                                      name="xnF")
                st_["qF"] = [sgp.tile([P, SW], BF16, tag=f"qF{m}",
                                      name=f"qF{m}") for m in range(KC)]
                st_["kF"] = [sgp.tile([P, SW], BF16, tag=f"kF{m}",
                                      name=f"kF{m}") for m in range(KC)]
                st_["vaug"] = sgp.tile([P, SEG, H, D + 1], BF16, tag="vaug",
                                       name="vaug")
                st_["attn"] = sgp.tile([P, SEG * C], BF16, tag="attn",
                                       name="attn")
                st_["xn2F"] = sgp.tile([P, KC * SW], FP8, tag="xn2F",
                                       name="xn2F")
                st_["x2"] = sgp.tile([P, SEG, C], F32, tag="x2", name="x2")
                st_["mvA"] = sgp.tile([P, SEG, 2], F32, tag="mvA", name="mvA")
                st_["rstdA"] = sgp.tile([P, SEG], F32, tag="rstdA",
                                        name="rstdA")
                st_["nmrA"] = sgp.tile([P, SEG], F32, tag="nmrA", name="nmrA")
                st_["mvD"] = sgp.tile([P, SEG, 2], F32, tag="mvD", name="mvD")
                st_["nmrD"] = sgp.tile([P, SEG], F32, tag="nmrD", name="nmrD")
                st_["rstdD"] = sgp.tile([P, SEG], F32, tag="rstdD",
                                        name="rstdD")
                return st_

            def emit_A_ln(st_, p):
                # pair-batched: one DMA + one bn_stats for tiles 2p, 2p+1;
                # the x tile stays resident for the residual add in emit_D.
                xp = wk.tile([P, 2, C], F32, tag="xa", bufs=10)
                i0 = st_["i0"]
                nc.sync.dma_start(
                    out=xp,
                    in_=x_d[(i0 + 2 * p) * P:(i0 + 2 * p + 2) * P, :]
                    .rearrange("(t p) c -> p t c", t=2))
                st2 = wk.tile([P, 2, 6], F32, tag="lnstats")
                ln_stats_pair(xp, st2, st_["mvA"], p)
                st_[f"xa_{p}"] = xp

            def emit_A_scal(st_, p):
                ln_scal(st_["mvA"], st_["rstdA"], p)
                # nmr = -mean * rstd (bias operand for the ACT-side normalize)
                nc.vector.scalar_tensor_tensor(
                    out=st_["nmrA"][:, 2 * p:2 * p + 2],
                    in0=st_["mvA"][:, 2 * p:2 * p + 2, 0], scalar=-1.0,
                    in1=st_["rstdA"][:, 2 * p:2 * p + 2],
                    op0=ALU.mult, op1=ALU.mult)

            def emit_A_xn(st_, t):
                xn0 = wk.tile([P, C], BF16, tag="lnxn0A", bufs=4, name="xn0")
                nc.scalar.activation(
                    out=xn0, in_=st_[f"xa_{t // 2}"][:, t % 2, :],
                    func=ACTF.Identity, bias=st_["nmrA"][:, t:t + 1],
                    scale=st_["rstdA"][:, t:t + 1])
                st_[f"xn0_{t}"] = xn0

            def emit_A_tp_pair(st_, u):
                tp_pair_to_F(u, st_.pop(f"xn0_{2 * u}"),
                             st_.pop(f"xn0_{2 * u + 1}"), st_["xnF"])

            def emit_B(st_):
                xnF, qF, kF = st_["xnF"], st_["qF"], st_["kF"]
                xnF_r = xnF.rearrange("p (k w) -> p k w", k=KC)
                for m in range(KC):
                    for g in range(ng):
                        for wsb, dstF, bias in ((wq_sb, qF, bq_sb),
                                                (wk_sb, kF, bk_sb)):
                            pqk = ps.tile([P, 512], F32, tag="st", bufs=2)
                            nc.tensor.matmul(
                                pqk, wsb[:, 0:2, m * P:(m + 1) * P],
                                xnF_r[:, 0:2, g * 512:(g + 1) * 512],
                                start=True, stop=False, perf_mode=DRM)
                            nc.tensor.matmul(
                                pqk, wsb[:, 2, m * P:(m + 1) * P],
                                xnF_r[:, 2, g * 512:(g + 1) * 512],
                                start=False, stop=True)
                            nc.scalar.activation(
                                out=dstF[m][:, g * 512:(g + 1) * 512],
                                in_=pqk, func=ACTF.Identity,
                                bias=bias[:, m:m + 1], scale=1.0)
                # (v projection emitted per-slot in emit_Bv: vaug[t]
                # is only needed at S1b slot t+2, and late emission keeps its
                # Tile priority below the critical scores chain)
                vaug = st_["vaug"]
                nc.vector.memset(vaug[:, :, :, D:D + 1], WS)

            def emit_Bv(st_, t):
                # v projection for tile t (xnF stationary -> T-layout; psum
                # holds 8v, so the softmax denominator column is 8 too)
                xnF_r = st_["xnF"].rearrange("p (k w) -> p k w", k=KC)
                vaug = st_["vaug"]
                pvf = ps.tile([P, 512], F32, tag="vf", bufs=2)
                pv = pvf[:, 0:C]
                nc.tensor.matmul(
                    pv, xnF_r[:, 0:2, t * P:(t + 1) * P],
                    wv_sb[:, 0:2, :], start=True, stop=False,
                    perf_mode=DRM)
                nc.tensor.matmul(
                    pv, xnF_r[:, 2, t * P:(t + 1) * P], wv_sb[:, 2, :],
                    start=False, stop=(not has_bv))
                if has_bv:
                    nc.tensor.matmul(pv, ones1, bv_sb, start=False,
                                     stop=True)
                _copy(t, vaug[:, t, :, 0:D],
                      pv.rearrange("p (h d) -> p h d", h=H))

            def emit_S1a(st_, t):
                qF, kF = st_["qF"], st_["kF"]
                # attention: S^T computed as full [128,128] blocks per
                # (head-parity, chunk): both sequences of the pair at once.
                # Cross-sequence quadrants are garbage that the block-diagonal
                # causal mask zeroes before AV. Two psum banks by head parity
                # (HW forbids mixed PE row-groups per bank partition range).
                # em columns: block (hp, ch) at (hp*KC + ch) * 128.
                # qF/kF hold 8q/8k, so exp scale has an extra /64.
                em = wk.tile([P, H * P], BF16, tag="em", bufs=5)
                for hp in range(2):
                    sthf = ps.tile([P, 512], F32, tag="st", bufs=2,
                                   name="sth")
                    sth = sthf[:, 0:KC * P]
                    pb = hp * 64
                    for ch in range(KC):
                        nc.tensor.matmul(
                            sth[:, ch * P:(ch + 1) * P],
                            kF[ch][pb:pb + 64, t * P:(t + 1) * P],
                            qF[ch][pb:pb + 64, t * P:(t + 1) * P],
                            start=True, stop=True)
                    nc.scalar.activation(
                        out=em[:, hp * KC * P:(hp + 1) * KC * P], in_=sth,
                        func=ACTF.Exp, bias=0.0, scale=SCALE / (WS * WS))
                nc.vector.tensor_mul(out=em, in0=em, in1=maskt)
                st_[f"em{t}"] = em

            def emit_S1b(st_, t):
                vaug, attn = st_["vaug"], st_["attn"]
                em = st_.pop(f"em{t}")
                avf = ps.tile([P, 512], F32, tag="avpr", bufs=2)
                av = avf[:, 0:H * (D + 1)].rearrange("p (h e) -> p h e",
                                                     e=D + 1)
                for ch in range(KC):
                    for hp in range(2):
                        h = 2 * ch + hp
                        bc = (hp * KC + ch) * P
                        nc.tensor.matmul(
                            av[:, h, :], em[:, bc:bc + P],
                            vaug[:, t, h, :], start=True, stop=True)
                invl = wk.tile([P, H], F32, tag="invl")
                nc.vector.reciprocal(
                    out=invl, in_=av[:, :, D:D + 1].rearrange("p h 1 -> p h"))
                nc.vector.tensor_mul(
                    out=attn[:, t * C:(t + 1) * C].rearrange(
                        "p (h d) -> p h d", h=H),
                    in0=av[:, :, 0:D],
                    in1=invl.unsqueeze(2).broadcast_to([P, H, D]))

            def emit_D_tile(st_, t):
                attn, x2 = st_["attn"], st_["x2"]
                # proj + residual (LN2 stats are pair-batched in emit_D_scal)
                tp = ps.tile([P, C], BF16, tag="big", bufs=2)
                for k in range(KC):
                    nc.tensor.transpose(
                        tp[:, k * P:(k + 1) * P],
                        attn[:, t * C + k * P: t * C + (k + 1) * P], ident)
                aoF = wk.tile([P, KC, P], FP8, tag="aoF")
                _copy(t, aoF, tp.rearrange("p (k c) -> p k c", c=P))
                pprf = ps.tile([P, 512], F32, tag="avpr", bufs=2)
                ppr = pprf[:, 0:C]
                nc.tensor.matmul(ppr, aoF[:, 0:2, :], wo_sb[:, 0:2, :],
                                 start=True, stop=False, perf_mode=DRM)
                nc.tensor.matmul(ppr, aoF[:, 2, :], wo_sb[:, 2, :],
                                 start=False, stop=(not has_bo))
                if has_bo:
                    nc.tensor.matmul(ppr, ones1, bo_sb, start=False, stop=True)
                # x2 = x + ppr/8 (fp8 weight descale folded in); x tile is
                # still resident from emit_A_ln — no second DMA.
                nc.vector.scalar_tensor_tensor(
                    out=x2[:, t, :], in0=ppr, scalar=RWS,
                    in1=st_[f"xa_{t // 2}"][:, t % 2, :],
                    op0=ALU.mult, op1=ALU.add)

            def emit_D_scal(st_, p):
                st2 = wk.tile([P, 2, 6], F32, tag="lnstats")
                ln_stats_pair(st_["x2"][:, 2 * p:2 * p + 2, :], st2,
                              st_["mvD"], p)
                ln_scal(st_["mvD"], st_["rstdD"], p)
                nc.vector.scalar_tensor_tensor(
                    out=st_["nmrD"][:, 2 * p:2 * p + 2],
                    in0=st_["mvD"][:, 2 * p:2 * p + 2, 0], scalar=-1.0,
                    in1=st_["rstdD"][:, 2 * p:2 * p + 2],
                    op0=ALU.mult, op1=ALU.mult)

            def emit_D3_xn(st_, t):
                xn0d = wk.tile([P, C], BF16, tag="lnxn0D", bufs=4, name="xn0d")
                nc.scalar.activation(
                    out=xn0d, in_=st_["x2"][:, t, :],
                    func=ACTF.Identity, bias=st_["nmrD"][:, t:t + 1],
                    scale=st_["rstdD"][:, t:t + 1])
                st_[f"xn0d_{t}"] = xn0d

            def emit_D3_tp(st_, u):
                tp_pair_to_F(u, st_.pop(f"xn0d_{2 * u}"),
                             st_.pop(f"xn0d_{2 * u + 1}"), st_["xn2F"],
                             eng_off=1)

            def emit_EF_chunk(st_, ph):
                # 8 phases per segment: per group g: E half-j, E half-j,
                # F tiles 0-1, F tiles 2-3
                g = ph // 4
                sub = ph % 4
                if sub == 0:
                    emit_E(st_, g, 0, JC // 2)
                elif sub == 1:
                    emit_E(st_, g, JC // 2, JC)
                elif sub == 2:
                    emit_F(st_, g, 0, 2)
                else:
                    emit_F(st_, g, 2, 4)

            def emit_E(st_, g, j0, j1):
                xn2F = st_["xn2F"]
                xn2F_r = xn2F.rearrange("p (k w) -> p k w", k=KC)
                hFg = st_.get(f"hF{g}")
                if hFg is None:
                    hFg = wk.tile([P, JC, 512], FP8, tag=f"hFall{g}", bufs=2,
                                  name=f"hF{g}")
                    st_[f"hF{g}"] = hFg
                for j in range(j0, j1):
                    phf = ps.tile([P, 512], F32, tag="big", bufs=2)
                    nc.tensor.matmul(
                        phf, w1_sb[:, 0:2, j * P:(j + 1) * P],
                        xn2F_r[:, 0:2, g * 512:(g + 1) * 512],
                        start=True, stop=False, perf_mode=DRM)
                    nc.tensor.matmul(
                        phf, w1_sb[:, 2, j * P:(j + 1) * P],
                        xn2F_r[:, 2, g * 512:(g + 1) * 512],
                        start=False, stop=True)
                    # psum holds 8*(xn2@W1); relu(s + 8bh) = 8*relu(s/8+bh),
                    # so hF stores 8h and W2's descale becomes /64 at the end.
                    # 3:1 ACT:DVE split (DVE carries the LN/softmax/residual)
                    if j % 4 != 3:
                        nc.scalar.activation(out=hFg[:, j, :], in_=phf,
                                             func=ACTF.Relu,
                                             bias=bh_sb[:, j:j + 1], scale=1.0)
                    else:
                        nc.vector.tensor_scalar(out=hFg[:, j, :], in0=phf,
                                                scalar1=bh_sb[:, j:j + 1],
                                                scalar2=0.0, op0=ALU.add,
                                                op1=ALU.max)

            def emit_F(st_, g, tg0, tg1):
                i0, x2 = st_["i0"], st_["x2"]
                hFg = st_[f"hF{g}"]
                otp = wk.tile([P, 2, C], F32, tag="ot")
                for tg in range(tg0, tg1):
                    t = g * (512 // P) + tg
                    pff = ps.tile([P, 512], F32, tag="vf", bufs=2)
                    pf = pff[:, 0:C]
                    for jp in range(JC // 2):
                        nc.tensor.matmul(
                            pf, hFg[:, 2 * jp:2 * jp + 2, tg * P:(tg + 1) * P],
                            w2_sb[:, 2 * jp:2 * jp + 2, :],
                            start=(jp == 0),
                            stop=(jp == JC // 2 - 1 and not has_b2),
                            perf_mode=DRM)
                    if has_b2:
                        nc.tensor.matmul(pf, ones1, b2_sb, start=False,
                                         stop=True)
                    # out = x2 + pf/64 (8h @ 8W2 descale); paired store
                    nc.vector.scalar_tensor_tensor(
                        out=otp[:, tg - tg0, :], in0=pf, scalar=RWS * RWS,
                        in1=x2[:, t, :], op0=ALU.mult, op1=ALU.add)
                t0 = g * (512 // P) + tg0
                nc.sync.dma_start(
                    out=out_d[(i0 + t0) * P:(i0 + t0 + 2) * P, :]
                    .rearrange("(t p) c -> p t c", t=2), in_=otp)

            # ====== software-pipelined emission over segments ======
            # While segment s runs attention/proj (latency-bound, PE-sparse),
            # the instruction streams also carry segment s+1's LN1 loads and
            # segment s-1's FFN groups (PE-dense) to keep every engine fed.
            def _emit_all():
                cur = new_state(0)
                for p in range(SEG // 2):
                    emit_A_ln(cur, p)
                    emit_A_scal(cur, p)
                for t in range(SEG):
                    emit_A_xn(cur, t)
                    if t % 2 == 1:
                        emit_A_tp_pair(cur, t // 2)
                prv = None
                for s in range(nseg):
                    emit_B(cur)
                    nxt = new_state(s + 1) if s + 1 < nseg else None
                    for t in range(SEG + 6):
                        if t < SEG:
                            emit_S1a(cur, t)     # S^T matmuls + exp
                            emit_Bv(cur, t)      # just-in-time v projection
                        if 2 <= t <= SEG + 1:
                            emit_S1b(cur, t - 2)  # mask, AV, normalize
                        if 3 <= t <= SEG + 2:
                            emit_D_tile(cur, t - 3)  # proj + residual
                        if t in (5, 7, 9, 11):
                            emit_D_scal(cur, (t - 5) // 2)  # LN2 pair stats
                        if 5 <= t <= SEG + 4:
                            emit_D3_xn(cur, t - 5)  # LN2 normalize
                        if t in (7, 9, 11, 13):
                            emit_D3_tp(cur, (t - 7) // 2)  # pair transposes
                        if nxt is not None:
                            if t < SEG and t % 2 == 0:
                                emit_A_ln(nxt, t // 2)
                            if t in (1, 3, 5, 7):
                                emit_A_scal(nxt, (t - 1) // 2)
                            if 2 <= t <= SEG + 1:
                                emit_A_xn(nxt, t - 2)
                            if t in (4, 6, 8, 10):
                                emit_A_tp_pair(nxt, (t - 4) // 2)
                        if prv is not None and t < 8:
                            emit_EF_chunk(prv, t)
                    prv, cur = cur, nxt
                for ph in range(8):
                    emit_EF_chunk(prv, ph)

            import contextlib
            loop_ctx = tc.For_i(0, loop_n) if loop_n > 1 \
                else contextlib.nullcontext()
            with loop_ctx:
                _emit_all()

    nc.compile()
    return nc


def _bf16(a):
    return np.asarray(a, np.float32).astype(ml_dtypes.bfloat16)


def _fp8(a, scale=1.0):
    a = np.asarray(a, np.float32) * scale
    return np.clip(a, -240.0, 240.0).astype(ml_dtypes.float8_e4m3)


def _prep(ln1_g, ln1_b, Wq, Wk, Wv, Wo, bo, ln2_g, ln2_b, W1, b1, W2, b2):
    """Host-side weight prep: fold LN affine into weights, scale by 8 for
    fp8-e4m3 range, pack aux consts."""
    ln1_g = np.asarray(ln1_g, np.float32)
    ln1_b = np.asarray(ln1_b, np.float32)
    ln2_g = np.asarray(ln2_g, np.float32)
    ln2_b = np.asarray(ln2_b, np.float32)
    wq_all = np.asarray(Wq, np.float32).transpose(1, 0, 2).reshape(C, C)
    wk_all = np.asarray(Wk, np.float32).transpose(1, 0, 2).reshape(C, C)
    wv_all = np.asarray(Wv, np.float32).transpose(1, 0, 2).reshape(C, C)
    W1 = np.asarray(W1, np.float32)
    bq = ln1_b @ wq_all
    bk = ln1_b @ wk_all
    bv = ln1_b @ wv_all
    bh = np.asarray(b1, np.float32) + ln2_b @ W1
    causal_t = np.tril(np.ones((T, T), np.float32)).T  # [s, t]: 1 iff s <= t
    mask_bd = np.zeros((P, P), np.float32)  # block-diag causal^T for seq pair
    mask_bd[:T, :T] = causal_t
    mask_bd[T:, T:] = causal_t
    d = {
        "wq": _fp8(ln1_g[:, None] * wq_all, WS),
        "wk": _fp8(ln1_g[:, None] * wk_all, WS),
        "wv": _fp8(ln1_g[:, None] * wv_all, WS),
        "wo": _fp8(np.asarray(Wo, np.float32), WS),
        "w1": _fp8(ln2_g[:, None] * W1, WS),
        "w2": _fp8(np.asarray(W2, np.float32), WS),
        "bq": (WS * bq).reshape(KC, P).T.copy(),
        "bk": (WS * bk).reshape(KC, P).T.copy(),
        "bh": (WS * bh).reshape(JC, P).T.copy(),
        "bv": _bf16(WS * bv).reshape(1, C),
        "bo_r": _bf16(WS * np.asarray(bo, np.float32)).reshape(1, C),
        "b2_r": _bf16(WS * WS * np.asarray(b2, np.float32)).reshape(1, C),
        "ident": np.eye(P, dtype=np.float32).astype(ml_dtypes.bfloat16),
        "maskt": _bf16(np.tile(mask_bd, (1, H))),
    }
    flags = (bool(np.any(bv != 0)), bool(np.any(np.asarray(bo) != 0)),
             bool(np.any(np.asarray(b2) != 0)))
    return d, flags


def kernel(x, ln1_g, ln1_b, Wq, Wk, Wv, Wo, bo, ln2_g, ln2_b, W1, b1, W2, b2):
    global last_exec_time_ns
    x = np.asarray(x, np.float32)
    aux, flags = _prep(ln1_g, ln1_b, Wq, Wk, Wv, Wo, bo, ln2_g, ln2_b, W1, b1,
                       W2, b2)
    key = flags
    if key not in _CACHE:
        _CACHE[key] = _build(*flags)
    nc = _CACHE[key]
    in_maps = []
    for c in range(N_CORES):
        m = dict(aux)
        m["x"] = x[c * B_LOC:(c + 1) * B_LOC].reshape(NTOK, C)
        in_maps.append(m)
    trace = bool(os.environ.get("BASS_TRACE"))
    try:
        res = run_bass_kernel_spmd(nc, in_maps, list(range(N_CORES)),
                                   trace=trace)
    except ModuleNotFoundError:
        res = run_bass_kernel_spmd(nc, in_maps, list(range(N_CORES)))
    last_exec_time_ns = res.exec_time_ns
    out = np.stack([res.results[c]["out"] for c in range(N_CORES)])
    return out.reshape(B_FULL, T, C).astype(np.float32)
